# revision 4
# baseline (speedup 1.0000x reference)
"""Trainium2 Bass kernel for nn_MemoryBank (retrieval_knn).

Strategy (K-sharded, NOT batch-sharded):
  Each of the 8 cores owns a 1/8 slice of the flattened (C,H,W) dim of every
  layer. Per core: partial score[b,m] = sum_l (m2_l[m] - 2*cross_l[b,m])/K_l
  over its k-slice (f^2 term dropped - constant in m, argmin-invariant),
  accumulated in one PSUM tile via PE matmuls; a tiny [32,30] AllReduce
  completes the score; argmin -> row indices -> indirect-DMA gather of the
  selected memory rows (this core's k-slice); (sel-f)^2 and the channel-concat
  outputs are written per-slice. Every mem byte is read once chip-wide.

Host-side pre-swizzle per core c (K4 = K_l/32, Kc = K_l/8, T = Kc/128):
  fN_l  [128, K4]   partition c4*32+b  <- f[b, c4*K4 + j]        (c4 in 0..3)
  mT_l  [128, T*30] partition p, col t*30+m <- mem[m, t*128+p]   (k on partitions)
  mN_l  [30, Kc]    natural slice (gather source only)
"""

import math
import os
import sys

import numpy as np

sys.path.insert(0, "/opt/trn_rl_repo")

B = 32
M = 30
NCORES = 8
SHAPES = [(64, 64, 64), (128, 32, 32), (256, 16, 16)]
KFULL = [c * h * w for (c, h, w) in SHAPES]          # 262144, 131072, 65536
KC = [k // NCORES for k in KFULL]                    # 32768, 16384, 8192
K4 = [k // 4 for k in KC]                            # 8192, 4096, 2048
T = [k // 128 for k in KC]                           # 256, 128, 64
BIG = 1000.0
CH = 2048          # gather / diff chunk cols
MT_CH = 1920       # memT chunk cols (64 t-blocks)
FN_CH = 2048       # f natural load chunk cols

_cached = {}


def _consts():
    iota = np.broadcast_to(np.arange(M, dtype=np.float32), (B, M)).copy()
    rep = np.zeros((B, 128), dtype=np.float32)
    for q in range(4):
        rep[np.arange(B), q * 32 + np.arange(B)] = 1.0
    qcol = (np.arange(128, dtype=np.float32) // 32).reshape(128, 1).copy()
    ident = np.eye(128, dtype=np.float32)
    return {"iota": iota, "rep": rep, "qcol": qcol, "ident": ident}


def build_program():
    import concourse.bacc as bacc
    import concourse.bass as bass
    import concourse.mybir as mybir
    import concourse.tile as tile

    nc = bacc.Bacc(
        "TRN2",
        target_bir_lowering=False,
        debug=False,
        num_devices=NCORES,
    )
    f32 = mybir.dt.float32
    i32 = mybir.dt.int32

    fN, mT, mN, outs = [], [], [], []
    for li in range(3):
        fN.append(nc.dram_tensor(f"fn{li}", [128, K4[li]], f32, kind="ExternalInput"))
        mT.append(
            nc.dram_tensor(f"mt{li}", [128, T[li] * M], f32, kind="ExternalInput")
        )
        mN.append(nc.dram_tensor(f"mn{li}", [M, KC[li]], f32, kind="ExternalInput"))
        outs.append(
            nc.dram_tensor(f"o{li}", [B, 2, KC[li]], f32, kind="ExternalOutput")
        )
    c_iota = nc.dram_tensor("c_iota", [B, M], f32, kind="ExternalInput")
    c_rep = nc.dram_tensor("c_rep", [B, 128], f32, kind="ExternalInput")
    c_qcol = nc.dram_tensor("c_qcol", [128, 1], f32, kind="ExternalInput")
    c_id = nc.dram_tensor("c_id", [128, 128], f32, kind="ExternalInput")

    with tile.TileContext(nc) as tc:
        with (
            tc.tile_pool(name="res", bufs=1) as res,          # resident tiles
            tc.tile_pool(name="mtp", bufs=3) as mtp,          # memT stream
            tc.tile_pool(name="selp", bufs=3) as selp,        # gathered rows
            tc.tile_pool(name="dstp", bufs=3) as dstp,        # diff^2 staging
            tc.tile_pool(name="scr", bufs=2) as scr,          # ACT square scratch
            tc.tile_pool(name="small", bufs=1) as small,
            tc.tile_pool(name="ps_t", bufs=4, space="PSUM") as ps_t,
            tc.tile_pool(name="ps_s", bufs=1, space="PSUM") as ps_s,
            tc.tile_pool(name="ps_i", bufs=1, space="PSUM") as ps_i,
            tc.tile_pool(name="dram", bufs=1, space="DRAM") as dram,
        ):
            # ---- constants -> SBUF
            id_sb = small.tile([128, 128], f32, tag="id")
            nc.sync.dma_start(out=id_sb[:], in_=c_id.ap())
            iota_sb = small.tile([B, M], f32, tag="iota")
            nc.sync.dma_start(out=iota_sb[:], in_=c_iota.ap())
            rep_sb = small.tile([B, 128], f32, tag="rep")
            nc.sync.dma_start(out=rep_sb[:], in_=c_rep.ap())
            qcol_sb = small.tile([128, 1], f32, tag="qcol")
            nc.sync.dma_start(out=qcol_sb[:], in_=c_qcol.ap())
            ones_sb = small.tile([128, B], f32, tag="ones")
            nc.gpsimd.memset(ones_sb[:], 1.0)

            # ---- resident tiles
            fn_sb = [res.tile([128, K4[li]], f32, tag=f"fn{li}", name=f"fn_sb{li}") for li in range(3)]
            ft_sb = [res.tile([128, T[li] * B], f32, tag=f"ft{li}", name=f"ft_sb{li}") for li in range(3)]
            msq = [small.tile([128, M], f32, tag=f"msq{li}", name=f"msq{li}") for li in range(3)]

            score_ps = ps_s.tile([B, M], f32)

            # ---- load f natural (chunked), write f-half of outputs
            for li in range(3):
                for c0 in range(0, K4[li], FN_CH):
                    c1 = min(c0 + FN_CH, K4[li])
                    nc.sync.dma_start(
                        out=fn_sb[li][:, c0:c1], in_=fN[li].ap()[:, c0:c1]
                    )
            out_v = [
                outs[li].ap().rearrange("b t (c j) -> c b t j", c=4)
                for li in range(3)
            ]
            for li in range(3):
                nc.sync.dma_start(out=out_v[li][:, :, 0, :], in_=fn_sb[li][:])

            # ---- transpose f blocks: fT[u, t4*128 + c4*32 + b]
            first_mm = True
            for li in range(3):
                t4n = T[li] // 4
                for t4 in range(t4n):
                    ftp = ps_t.tile([128, 128], f32, tag="ftp")
                    nc.tensor.transpose(
                        out=ftp[:],
                        in_=fn_sb[li][:, t4 * 128 : (t4 + 1) * 128],
                        identity=id_sb[:],
                    )
                    nc.vector.tensor_scalar(
                        out=ft_sb[li][:, t4 * 128 : (t4 + 1) * 128],
                        in0=ftp[:],
                        scalar1=-2.0 / KFULL[li],
                        scalar2=None,
                        op0=mybir.AluOpType.mult,
                    )

            # ---- stream memT: cross matmuls + m2 squares/reduction
            for li in range(3):
                t4n = T[li] // 4
                cols = T[li] * M
                nc.gpsimd.memset(msq[li][:], 0.0)
                for c0 in range(0, cols, MT_CH):
                    c1 = min(c0 + MT_CH, cols)
                    nt = (c1 - c0) // M
                    t0 = c0 // M
                    mt_t = mtp.tile([128, MT_CH], f32, tag="mt")
                    nc.sync.dma_start(out=mt_t[:, : c1 - c0], in_=mT[li].ap()[:, c0:c1])
                    # cross: one matmul per t-block in this chunk
                    for tt_ in range(nt):
                        t = t0 + tt_
                        c4, t4 = t // t4n, t % t4n
                        lhsT = ft_sb[li][:, t4 * 128 + c4 * 32 : t4 * 128 + c4 * 32 + 32]
                        nc.tensor.matmul(
                            score_ps[:],
                            lhsT,
                            mt_t[:, tt_ * M : (tt_ + 1) * M],
                            start=first_mm,
                            stop=False,
                        )
                        first_mm = False
                    # m2: square (ACT) then per-m sum over t (strided reduce)
                    sq_t = scr.tile([128, MT_CH], f32, tag="sq")
                    nc.scalar.square(out=sq_t[:, : c1 - c0], in_=mt_t[:, : c1 - c0])
                    red = small.tile([128, M], f32, tag="red")
                    nc.vector.tensor_reduce(
                        out=red[:],
                        in_=sq_t[:, : c1 - c0].rearrange("p (t m) -> p m t", m=M),
                        op=mybir.AluOpType.add,
                        axis=mybir.AxisListType.X,
                    )
                    nc.vector.tensor_tensor(
                        out=msq[li][:], in0=msq[li][:], in1=red[:],
                        op=mybir.AluOpType.add,
                    )
                nc.vector.tensor_scalar(
                    out=msq[li][:], in0=msq[li][:], scalar1=1.0 / KFULL[li],
                    scalar2=None, op0=mybir.AluOpType.mult,
                )
                nc.tensor.matmul(
                    score_ps[:],
                    ones_sb[:],
                    msq[li][:],
                    start=False,
                    stop=(li == 2),
                )

            # ---- AllReduce the partial score
            score_sb = small.tile([B, M], f32, tag="scsb")
            nc.vector.tensor_copy(out=score_sb[:], in_=score_ps[:])
            cc_in = dram.tile([B, M], f32)
            cc_out = dram.tile([B, M], f32)
            nc.sync.dma_start(out=cc_in[:], in_=score_sb[:])
            nc.gpsimd.collective_compute(
                "AllReduce",
                mybir.AluOpType.add,
                replica_groups=[list(range(NCORES))],
                ins=[cc_in[:]],
                outs=[cc_out[:]],
            )
            score_f = small.tile([B, M], f32, tag="scf")
            nc.sync.dma_start(out=score_f[:], in_=cc_out[:])

            # ---- argmin -> first-match index (f32)
            rmin = small.tile([B, 1], f32, tag="rmin")
            nc.vector.tensor_reduce(
                out=rmin[:], in_=score_f[:], op=mybir.AluOpType.min,
                axis=mybir.AxisListType.X,
            )
            eq = small.tile([B, M], f32, tag="eq")
            nc.vector.tensor_scalar(
                out=eq[:], in0=score_f[:], scalar1=rmin[:], scalar2=None,
                op0=mybir.AluOpType.is_equal,
            )
            u1 = small.tile([B, M], f32, tag="u1")
            nc.vector.tensor_tensor(
                out=u1[:], in0=eq[:], in1=iota_sb[:], op=mybir.AluOpType.mult
            )
            u2 = small.tile([B, M], f32, tag="u2")
            nc.vector.tensor_scalar(
                out=u2[:], in0=eq[:], scalar1=-BIG, scalar2=BIG,
                op0=mybir.AluOpType.mult, op1=mybir.AluOpType.add,
            )
            nc.vector.tensor_tensor(
                out=u1[:], in0=u1[:], in1=u2[:], op=mybir.AluOpType.add
            )
            idxf = small.tile([B, 1], f32, tag="idxf")
            nc.vector.tensor_reduce(
                out=idxf[:], in_=u1[:], op=mybir.AluOpType.min,
                axis=mybir.AxisListType.X,
            )

            # ---- offsets[q*32+b] = 4*idx[b] + q  (int32)
            idxq_ps = ps_i.tile([128, 1], f32)
            nc.tensor.matmul(idxq_ps[:], rep_sb[:], idxf[:], start=True, stop=True)
            idx4 = small.tile([128, 1], f32, tag="idx4")
            nc.vector.tensor_scalar(
                out=idx4[:], in0=idxq_ps[:], scalar1=4.0, scalar2=None,
                op0=mybir.AluOpType.mult,
            )
            nc.vector.tensor_tensor(
                out=idx4[:], in0=idx4[:], in1=qcol_sb[:], op=mybir.AluOpType.add
            )
            off_i = small.tile([128, 1], i32, tag="offi")
            nc.vector.tensor_copy(out=off_i[:], in_=idx4[:])

            # ---- gather selected rows + (sel-f)^2 + diff-half output
            for li in range(3):
                for c0 in range(0, K4[li], CH):
                    c1 = min(c0 + CH, K4[li])
                    w = c1 - c0
                    sel_t = selp.tile([128, CH], f32, tag="sel")
                    nc.gpsimd.indirect_dma_start(
                        out=sel_t[:, :w],
                        out_offset=None,
                        in_=mN[li].ap().rearrange("m (x j) -> (m x) j", j=K4[li]),
                        in_offset=bass.IndirectOffsetOnAxis(ap=off_i[:, :1], axis=0),
                        element_offset=c0,
                    )
                    d_t = dstp.tile([128, CH], f32, tag="dst")
                    nc.vector.tensor_tensor(
                        out=d_t[:, :w], in0=sel_t[:, :w], in1=fn_sb[li][:, c0:c1],
                        op=mybir.AluOpType.subtract,
                    )
                    nc.scalar.square(out=d_t[:, :w], in_=d_t[:, :w])
                    nc.sync.dma_start(
                        out=out_v[li][:, :, 1, c0:c1], in_=d_t[:, :w]
                    )

    nc.compile()
    return nc


def _shard_inputs(inputs):
    """-> (in_maps, meta). inputs keyed f1..f3, mem1..mem3 (full shapes)."""
    consts = _consts()
    fs = [np.ascontiguousarray(inputs[f"f{i+1}"], dtype=np.float32).reshape(B, -1)
          for i in range(3)]
    ms = [np.ascontiguousarray(inputs[f"mem{i+1}"], dtype=np.float32).reshape(M, -1)
          for i in range(3)]
    in_maps = []
    for c in range(NCORES):
        d = dict(c_iota=consts["iota"], c_rep=consts["rep"],
                 c_qcol=consts["qcol"], c_id=consts["ident"])
        for li in range(3):
            kc, k4, t = KC[li], K4[li], T[li]
            fsl = fs[li][:, c * kc : (c + 1) * kc]                    # [B, Kc]
            d[f"fn{li}"] = np.ascontiguousarray(
                fsl.reshape(B, 4, k4).transpose(1, 0, 2).reshape(128, k4)
            )
            msl = ms[li][:, c * kc : (c + 1) * kc]                    # [M, Kc]
            d[f"mt{li}"] = np.ascontiguousarray(
                msl.reshape(M, t, 128).transpose(2, 1, 0).reshape(128, t * M)
            )
            d[f"mn{li}"] = np.ascontiguousarray(msl)
        in_maps.append(d)
    return in_maps


def _assemble(results):
    outs = []
    for li in range(3):
        kf, kc = KFULL[li], KC[li]
        full = np.empty((B, 2 * kf), dtype=np.float32)
        for c in range(NCORES):
            o = results[c][f"o{li}"]
            full[:, c * kc : (c + 1) * kc] = o[:, 0, :]
            full[:, kf + c * kc : kf + (c + 1) * kc] = o[:, 1, :]
        C, H, W = SHAPES[li]
        outs.append(full.reshape(B, 2 * C, H, W))
    return tuple(outs)


def kernel(**inputs):
    from concourse.bass_utils import run_bass_kernel_spmd

    if "nc" not in _cached:
        _cached["nc"] = build_program()
    in_maps = _shard_inputs(inputs)
    res = run_bass_kernel_spmd(
        _cached["nc"], in_maps, list(range(NCORES)), **_cached.get("run_kwargs", {})
    )
    _cached["last_results"] = res
    return _assemble(res.results)


if __name__ == "__main__":
    # smoke: build only
    nc = build_program()
    print("build ok")
